# revision 52
# baseline (speedup 1.0000x reference)
"""Routed per-behavior FFN (MoE-style) Trainium2 kernel.

Reference semantics: for each token t with b = b_seq[t]:
  b == 0      -> output 0
  b in 1..4   -> LN(elu(x W1_b^T + b1_b) W2_b^T + b2_b) * gamma_b + beta_b

Strategy (~127us vs the 154us v1 baseline; engines ACT/DVE/PE all ~95%
balanced at ~97us busy each):
- Host routing (metadata only): tokens sorted by branch, each branch's
  token list split evenly over 8 cores; every core runs an identical-shape
  grouped FFN over ~1/8 of the routed tokens. bf16 matmuls, f32 PSUM.
- ELU with NO +1 offset: u' = min(e^ph,1) - 1 in ONE dual-op
  tensor_scalar (4x bf16 mode), then h = max(ph, u') = elu(ph) exactly
  (e^v - 1 >= v) via one STT from PSUM. Host pre-centers W2 columns
  (and b2), so the W2 matmul output py is already LN-centered: no mean
  column, no cvec, no center op. Small chunks pack all 8 f-chunks into
  one ph tile (viewed [128, FG, W]) to pay one PE->ACT->DVE round trip.
- Software pipeline, one chunk deep: the y-stage of chunk c (W2 matmuls
  + ACT Square-with-accum_out variance + ACT Copy evacuation of py ->
  SBUF bf16) is emitted during chunk c+1's h-stage, so W2(c) interleaves
  with W1(c+1) on the PE and the ACT ops never head-block. A few squares
  per chunk run on DVE instead (engine balance), deferred one chunk
  further, together with the batched rstd (quake bit-trick + 1 Newton
  step, ~6 tiny DVE ops per 2 chunks) so they land where the DVE idles.
- LN apply deferred by one branch: o2 = yc * rstd via a broadcast-AP
  tensor_tensor on the otherwise-idle GPSIMD engine (~0.6us/tile, a full
  branch of slack), stores on the sync queue. The final branch drains
  3 units on DVE + 1 on GPSIMD in parallel.
- Weight preloads on gpsimd+scalar DMA queues (never ACT-queue compute
  time mid-kernel; a trigger costs ~0.8us of issuing-engine time at t=0
  when those engines are idle). Branch 0's weights are split f-leading
  across both queues so the first matmul starts ~5us after the DMA
  rings come up.
"""

import json

import numpy as np

B, T = 32, 2048
D_MODEL = 256
D_FF = 1024
N_B = 4
NCORES = 8
NTOK = B * T
QUAKE_MAGIC = 0x5F3759DF + 0x02000000  # rsqrt(v/256): fold 1/256 into exponent

# ---------------------------------------------------------------------------
# walrus workaround: this container's compiler accepts at most one sync wait
# per CTRL-class instruction; split extras onto NoOp carriers.
# ---------------------------------------------------------------------------


def _split_excess_waits(bir: dict, max_waits: int = 1) -> None:
    for fn in bir.get("functions", []):
        for blk in fn.get("blocks", []):
            insts = blk.get("instructions")
            if not insts:
                continue
            new = []
            for inst in insts:
                si = inst.get("sync_info")
                waits = (si or {}).get("on_wait") or []
                if len(waits) > max_waits:
                    excess, keep = waits[:-max_waits], waits[-max_waits:]
                    for k, w in enumerate(excess):
                        new.append(
                            {
                                "debug": inst.get("debug", 0),
                                "engine": inst["engine"],
                                "ins": [],
                                "name": f"{inst['name']}-wsplit{k}",
                                "opcode": "NoOp",
                                "outs": [],
                                "sync_info": {"on_update": [], "on_wait": [w]},
                            }
                        )
                    si["on_wait"] = keep
                new.append(inst)
            blk["instructions"] = new


_bir_fix_installed = False


def _install_bir_fix():
    global _bir_fix_installed
    if _bir_fix_installed:
        return
    import concourse.bass_utils as bass_utils
    import concourse.bass2jax as bass2jax

    orig = bass_utils.compile_bir_kernel

    import os as _os

    if _os.environ.get("LDW_OPT"):
        _orig_bvo = bass_utils.bir_verify_and_optimise

        def _bvo(tmpdir, inp="bir.json", outp="file.neff", arch=None, **kw):
            import unittest.mock as _mock

            real_run = bass_utils.run_command

            def run2(argv, **kwargs):
                argv = [
                    a.replace("--enable-ldw-opt=false", "--enable-ldw-opt=true")
                    for a in argv
                ]
                return real_run(argv, **kwargs)

            with _mock.patch.object(bass_utils, "run_command", run2):
                return _orig_bvo(tmpdir, inp, outp, arch, **kw)

        bass_utils.bir_verify_and_optimise = _bvo

    def patched(bir_json, tmpdir, neff_name="file.neff"):
        bir = json.loads(bir_json)
        _split_excess_waits(bir)
        return orig(json.dumps(bir).encode(), tmpdir, neff_name)

    bass_utils.compile_bir_kernel = patched
    bass2jax.compile_bir_kernel = patched

    # Synthesize antenv.axon_hooks (absent in this image) so trace=True can
    # reach the terminal's NTFF profiler via the axon .so.
    import sys
    import types

    if "antenv.axon_hooks" not in sys.modules:
        try:
            from trn_agent_boot.trn_boot import _ntff_profile_via_ctypes

            hook = _ntff_profile_via_ctypes("/opt/axon/libaxon_pjrt.so")
            mod = types.ModuleType("antenv.axon_hooks")
            mod.get_axon_ntff_profile_hook = lambda: hook
            mod.set_axon_ntff_profile_hook = lambda h: None
            sys.modules["antenv.axon_hooks"] = mod
        except Exception:
            pass
    _bir_fix_installed = True


# ---------------------------------------------------------------------------
# device kernel builder
# ---------------------------------------------------------------------------

_BUILD_CACHE = {}


def _chunks(cap, w=512, small_first=False):
    out = []
    off = 0
    if small_first and cap % w:
        out.append((0, cap % w))
        off = cap % w
    while off < cap:
        out.append((off, min(w, cap - off)))
        off += w
    return out


def _build(caps, b1_nonzero, b2_nonzero, gb_nontrivial):
    key = (tuple(caps), b1_nonzero, b2_nonzero, gb_nontrivial)
    if key in _BUILD_CACHE:
        return _BUILD_CACHE[key]

    import concourse.bass as bass
    import concourse.tile as tile
    from concourse import mybir

    f32 = mybir.dt.float32
    bf16 = mybir.dt.bfloat16
    i32 = mybir.dt.int32
    KD = D_MODEL // 128  # 2 chunks of the model dim
    KF = D_FF // 128  # 8 chunks of the ff dim

    nc = bass.Bass("TRN2")
    S = sum(caps)
    NTILES = S // 128
    xg = nc.dram_tensor("xg", [KD, 128, S], bf16, kind="ExternalInput")
    w1t = nc.dram_tensor("w1t", [N_B, KD, 128, D_FF], bf16, kind="ExternalInput")
    w2t = nc.dram_tensor("w2t", [N_B, KF, 128, D_MODEL], bf16, kind="ExternalInput")
    if b2_nonzero:
        # centered b2, added via a rank-1 matmul
        ncvd = nc.dram_tensor("ncv", [N_B, D_MODEL], bf16, kind="ExternalInput")
    if b1_nonzero:
        b1d = nc.dram_tensor("b1", [N_B, D_FF], bf16, kind="ExternalInput")
    if gb_nontrivial:
        gamd = nc.dram_tensor("gamma", [N_B, D_MODEL], f32, kind="ExternalInput")
        betd = nc.dram_tensor("beta", [N_B, D_MODEL], f32, kind="ExternalInput")
    yc = nc.dram_tensor("yc", [S, D_MODEL], bf16, kind="ExternalOutput")

    AF = mybir.ActivationFunctionType
    OP = mybir.AluOpType

    with tile.TileContext(nc) as tc:
        with (
            tc.tile_pool(name="singles", bufs=1) as singles,
            tc.tile_pool(name="w1p", bufs=N_B) as w1p,
            tc.tile_pool(name="w2p", bufs=N_B) as w2p,
            tc.tile_pool(name="cns", bufs=2) as cns,
            tc.tile_pool(name="xp", bufs=4) as xp,
            tc.tile_pool(name="ep", bufs=4) as ep,
            tc.tile_pool(name="up", bufs=4) as up,
            tc.tile_pool(name="hp", bufs=3) as hp,
            tc.tile_pool(name="sqp", bufs=2) as sqp,
            tc.tile_pool(name="o2p", bufs=4) as o2p,
            tc.tile_pool(name="stp", bufs=8) as stp,
            tc.tile_pool(name="tsp", bufs=4) as tsp,
            tc.tile_pool(name="php", bufs=3, space="PSUM") as php,
            tc.tile_pool(name="pyp", bufs=2, space="PSUM") as pyp,
        ):
            # big staging buffer: un-normalized LN numerator for every token
            ycb = singles.tile([128, NTILES, D_MODEL], bf16)
            ones_col = singles.tile([1, 128], bf16)
            nc.vector.memset(ones_col, 1.0)
            if b1_nonzero:
                ones_row = singles.tile([1, 512], bf16)
                nc.vector.memset(ones_row, 1.0)

            # Preload every branch's weights up-front. Branch 0 is split
            # across three idle DMA queues (gpsimd/tensor/vector) so the
            # first matmul can start as soon as possible; the rest stream
            # on the gpsimd queue. Nothing goes on the ACT or sync queues.
            w1_sbs, w2_sbs = [], []
            for n in range(N_B):
                w1_sb = w1p.tile([128, KD, D_FF], bf16, tag="w1")
                w2_sb = w2p.tile([128, KF, D_MODEL], bf16, tag="w2")
                if n == 0:
                    # split the critical first weights across two idle queues,
                    # f-leading slice first (the first matmuls need only
                    # f 0:256). Scalar-queue triggers cost ~0.8us of ACT each
                    # at t=0, but ACT is idle until ~8us anyway.
                    nc.gpsimd.dma_start(
                        out=w1_sb[:, 0, :128], in_=w1t[n, 0, :, :128]
                    )
                    nc.scalar.dma_start(
                        out=w1_sb[:, 1, :128], in_=w1t[n, 1, :, :128]
                    )
                    nc.gpsimd.dma_start(
                        out=w1_sb[:, 0, 128:512], in_=w1t[n, 0, :, 128:512]
                    )
                    nc.scalar.dma_start(
                        out=w1_sb[:, 1, 128:512], in_=w1t[n, 1, :, 128:512]
                    )
                    nc.gpsimd.dma_start(
                        out=w1_sb[:, 0, 512:], in_=w1t[n, 0, :, 512:]
                    )
                    nc.scalar.dma_start(
                        out=w1_sb[:, 1, 512:], in_=w1t[n, 1, :, 512:]
                    )
                    nc.gpsimd.dma_start(
                        out=w2_sb[:, : KF // 2, :],
                        in_=w2t[n, : KF // 2].rearrange("j p d -> p j d"),
                    )
                    nc.scalar.dma_start(
                        out=w2_sb[:, KF // 2 :, :],
                        in_=w2t[n, KF // 2 :].rearrange("j p d -> p j d"),
                    )
                else:
                    for k in range(KD):
                        nc.gpsimd.dma_start(out=w1_sb[:, k, :], in_=w1t[n, k])
                    nc.gpsimd.dma_start(
                        out=w2_sb, in_=w2t[n].rearrange("j p d -> p j d")
                    )
                w1_sbs.append(w1_sb)
                w2_sbs.append(w2_sb)

            live = [n for n in range(N_B) if caps[n] > 0]

            def emit_rstd(varall, rst, lo, hi):
                # rst[:, lo:hi] = 1/sqrt(varall[:, lo:hi]/256)
                # quake seed (2 fused ops) + 1 Newton step (4 ops).
                nn = hi - lo
                t1 = tsp.tile([128, 8], f32, tag="t1")
                t2 = tsp.tile([128, 8], f32, tag="t2")
                nc.vector.tensor_scalar(
                    t1[:, :nn].bitcast(i32),
                    varall[:, lo:hi].bitcast(i32),
                    scalar1=1,
                    scalar2=-1,
                    op0=OP.logical_shift_right,
                    op1=OP.bitwise_xor,
                )
                nc.vector.tensor_scalar(
                    rst[:, lo:hi].bitcast(i32),
                    t1[:, :nn].bitcast(i32),
                    scalar1=QUAKE_MAGIC + 1,
                    scalar2=None,
                    op0=OP.add,
                )
                nc.vector.tensor_tensor(
                    t1[:, :nn], rst[:, lo:hi], rst[:, lo:hi], op=OP.mult
                )
                nc.vector.scalar_tensor_tensor(
                    t2[:, :nn],
                    varall[:, lo:hi],
                    -0.5 / 256.0,
                    t1[:, :nn],
                    op0=OP.mult,
                    op1=OP.mult,
                )
                nc.vector.tensor_scalar(
                    t2[:, :nn], t2[:, :nn], scalar1=1.5, scalar2=None, op0=OP.add
                )
                nc.vector.tensor_tensor(
                    rst[:, lo:hi], rst[:, lo:hi], t2[:, :nn], op=OP.mult
                )

            def emit_tail(unit, on_dve=False):
                # deferred LN apply + store for one earlier chunk. Normally on
                # the idle GPSIMD engine (~0.6us/tile, a branch of slack); the
                # final drain splits across DVE+GPSIMD to shorten the tail.
                rst, gb, goff, off, W = unit
                nW = W // 128
                o2 = o2p.tile([128, 4, D_MODEL], bf16, tag="o2")
                for t in range(nW):
                    tt = off // 128 + t
                    gtile = goff // 128 + t
                    if on_dve:
                        nc.vector.tensor_scalar_mul(
                            o2[:, t, :],
                            ycb[:, gtile, :],
                            scalar1=rst[:, tt : tt + 1],
                        )
                    else:
                        rcol = rst[:, tt : tt + 1]
                        rbc = bass.AP(
                            tensor=rcol.tensor,
                            offset=rcol.offset,
                            ap=[rcol.ap[0], [0, D_MODEL]],
                        )
                        nc.gpsimd.tensor_tensor(
                            o2[:, t, :], ycb[:, gtile, :], rbc, op=OP.mult
                        )
                    if gb_nontrivial:
                        gam_bc, bet_bc = gb
                        eng = nc.vector if on_dve else nc.gpsimd
                        eng.tensor_mul(o2[:, t, :], o2[:, t, :], gam_bc)
                        eng.tensor_add(o2[:, t, :], o2[:, t, :], bet_bc)
                nc.sync.dma_start(
                    out=yc[goff : goff + W, :].rearrange("(c p) d -> p c d", p=128),
                    in_=o2[:, :nW, :],
                )

            pending = []  # tail units of the previous branch
            rstd_pend = []  # rstd batches awaiting emission at a chunk top
            dve_pend = []  # deferred DVE squares (one chunk behind y-stage)
            y_pend = []  # deferred y-stage of the previous chunk
            seg_off = 0
            for n in live:
                cap = caps[n]
                w1_sb, w2_sb = w1_sbs[n], w2_sbs[n]
                if b2_nonzero:
                    ncv_sb = cns.tile([1, D_MODEL], bf16, tag="ncv")
                    nc.gpsimd.dma_start(out=ncv_sb, in_=ncvd[n : n + 1, :])
                if b1_nonzero:
                    b1_sb = cns.tile([1, D_FF], bf16, tag="b1")
                    nc.gpsimd.dma_start(out=b1_sb, in_=b1d[n : n + 1, :])
                gb = None
                if gb_nontrivial:
                    gam_bc = cns.tile([128, D_MODEL], f32, tag="gam")
                    bet_bc = cns.tile([128, D_MODEL], f32, tag="bet")
                    gsrc = gamd[n : n + 1, :]
                    bsrc = betd[n : n + 1, :]
                    nc.gpsimd.dma_start(
                        out=gam_bc,
                        in_=bass.AP(
                            tensor=gsrc.tensor,
                            offset=gsrc.offset,
                            ap=[[0, 128], gsrc.ap[1]],
                        ),
                    )
                    nc.gpsimd.dma_start(
                        out=bet_bc,
                        in_=bass.AP(
                            tensor=bsrc.tensor,
                            offset=bsrc.offset,
                            ap=[[0, 128], bsrc.ap[1]],
                        ),
                    )
                    gb = (gam_bc, bet_bc)

                NT = cap // 128
                varall = stp.tile([128, NT], f32, tag="var")
                rst = stp.tile([128, NT], f32, tag="rst")
                mytails = []
                pend_cols = []  # chunk column ranges awaiting a batched rstd
                first = n == live[0]
                for ci, (off, W) in enumerate(_chunks(cap, small_first=first)):
                    goff = seg_off + off
                    nW = W // 128
                    # deferred DVE squares then rstd batches: they land in
                    # the DVE FIFO where it would otherwise idle awaiting
                    # this chunk's first exp (never at a branch tail, where
                    # they head-block the next branch)
                    for fn in dve_pend:
                        fn()
                    del dve_pend[:]
                    for batch in rstd_pend:
                        emit_rstd(*batch)
                    del rstd_pend[:]
                    xg_sb = xp.tile([128, KD, 512], bf16, tag="xg")
                    for k in range(KD):
                        nc.sync.dma_start(
                            out=xg_sb[:, k, :W], in_=xg[k, :, goff : goff + W]
                        )
                    # ---- h-stage: h = elu(W1 x (+b1)) ----
                    # u' = min(e^ph,1) - 1 (dual-op ts, 4x), then
                    # h = max(ph, u') = elu(ph) exactly (e^v - 1 >= v).
                    # Small chunks pack MORE f-chunks per ph tile (same
                    # 1024 f32/partition viewed [128, FG, W]) so they pay
                    # one PE->ACT->DVE round trip instead of four.
                    FG = min(KF, max(2, 1024 // W))
                    h_sb = hp.tile([128, KF, 512], bf16, tag="h")

                    def fv(t, FG=FG, W=W):
                        return bass.AP(
                            tensor=t.tensor,
                            offset=t.offset,
                            ap=[t.ap[0], [W, FG], [1, W]],
                        )

                    for g in range(KF // FG):
                        ph = php.tile([128, 2, 512], f32, tag="ph")
                        phv = fv(ph)
                        for j in range(FG):
                            f = g * FG + j
                            fs = slice(f * 128, (f + 1) * 128)
                            nc.tensor.matmul(
                                phv[:, j, :],
                                w1_sb[:, 0, fs],
                                xg_sb[:, 0, :W],
                                start=True,
                                stop=False,
                            )
                            nc.tensor.matmul(
                                phv[:, j, :],
                                w1_sb[:, 1, fs],
                                xg_sb[:, 1, :W],
                                start=False,
                                stop=not b1_nonzero,
                            )
                            if b1_nonzero:
                                nc.tensor.matmul(
                                    phv[:, j, :],
                                    b1_sb[:, fs],
                                    ones_row[:, :W],
                                    start=False,
                                    stop=True,
                                )
                        e_sb = ep.tile([128, 2, 512], bf16, tag="e")
                        ev = fv(e_sb)
                        nc.scalar.activation(ev, phv, AF.Exp)
                        # u' = min(e^ph, 1) - 1  (dual-op ts, 4x mode)
                        u_sb = up.tile([128, 2, 512], bf16, tag="u")
                        uv = fv(u_sb)
                        nc.vector.tensor_scalar(
                            uv,
                            ev,
                            scalar1=1.0,
                            scalar2=-1.0,
                            op0=OP.min,
                            op1=OP.add,
                        )
                        # h = max(ph, u') = elu(ph) (exact: e^v-1 >= v)
                        nc.vector.scalar_tensor_tensor(
                            h_sb[:, g * FG : (g + 1) * FG, :W],
                            phv,
                            1.0,
                            uv,
                            op0=OP.mult,
                            op1=OP.max,
                        )
                    # ---- y-stage, deferred one chunk (software pipeline):
                    #      W2(c) interleaves with W1(c+1) on the PE, ACT
                    #      sq/cp never wait, DVE square + rstd defer one
                    #      chunk further ----
                    def make_y(
                        h_sb=h_sb,
                        off=off,
                        W=W,
                        goff=goff,
                        ci=ci,
                        varall=varall,
                        rst=rst,
                        pcols=pend_cols,
                        mt=mytails,
                        w2_sb=w2_sb,
                        gb=gb,
                        ncv_l=(ncv_sb if b2_nonzero else None),
                    ):
                        def run():
                            nW = W // 128
                            for t in range(nW):
                                sub = t % 2
                                if sub == 0:
                                    py = pyp.tile(
                                        [128, 2, D_MODEL], f32, tag="py"
                                    )
                                    psub = min(2, nW - t)
                                tt = off // 128 + t
                                for f in range(KF):
                                    nc.tensor.matmul(
                                        py[:, sub, :],
                                        h_sb[:, f, t * 128 : (t + 1) * 128],
                                        w2_sb[:, f, :],
                                        start=(f == 0),
                                        stop=not b2_nonzero and f == KF - 1,
                                    )
                                if b2_nonzero:
                                    nc.tensor.matmul(
                                        py[:, sub, :],
                                        ones_col[:, :128],
                                        ncv_l,
                                        start=False,
                                        stop=True,
                                    )
                                if not (t == 0 and ci in (0, 2)):
                                    # 256*var via ACT Square + row-accumulate
                                    sqs = sqp.tile(
                                        [128, D_MODEL], bf16, tag="sq"
                                    )
                                    nc.scalar.activation(
                                        sqs,
                                        py[:, sub, :],
                                        AF.Square,
                                        accum_out=varall[:, tt : tt + 1],
                                    )
                                if sub == psub - 1:
                                    # evacuate pair of tiles PSUM->SBUF bf16
                                    base = goff // 128 + t - sub
                                    nc.scalar.activation(
                                        ycb[:, base : base + psub, :],
                                        py[:, :psub, :],
                                        AF.Copy,
                                    )
                            if ci in (0, 2):
                                # balance: tile 0's square on DVE from the
                                # copy, deferred one further chunk
                                def dve_sq(
                                    varall=varall,
                                    tt0=off // 128,
                                    g0=goff // 128,
                                ):
                                    sqs = sqp.tile(
                                        [128, D_MODEL], bf16, tag="sq"
                                    )
                                    nc.vector.scalar_tensor_tensor(
                                        sqs,
                                        ycb[:, g0, :],
                                        0.0,
                                        ycb[:, g0, :],
                                        op0=OP.bypass,
                                        op1=OP.mult,
                                        accum_out=varall[:, tt0 : tt0 + 1],
                                    )

                                dve_pend.append(dve_sq)
                            pcols.append((off // 128, off // 128 + nW))
                            # batched rstd every other chunk (amortizes the
                            # tiny-op overhead of quake+Newton)
                            if len(pcols) >= 2:
                                rstd_pend.append(
                                    (varall, rst, pcols[0][0], pcols[-1][1])
                                )
                                del pcols[:]
                            mt.append((rst, gb, goff, off, W))

                        return run

                    y_pend.append(make_y())
                    if len(y_pend) > 1:
                        y_pend.pop(0)()
                    # deferred apply+store for one chunk of the previous branch
                    if pending:
                        emit_tail(pending.pop(0))

                # drain any leftovers before rebinding (unequal chunk counts)
                while pending:
                    emit_tail(pending.pop(0))
                pending = mytails
                seg_off += cap
            # kernel end: flush the pipeline, then drain the final tails
            # (~3 units on DVE, 1 on GPSIMD, in parallel)
            while y_pend:
                y_pend.pop(0)()
            for fn in dve_pend:
                fn()
            del dve_pend[:]
            for batch in rstd_pend:
                emit_rstd(*batch)
            del rstd_pend[:]
            for i, unit in enumerate(pending):
                emit_tail(unit, on_dve=(i != 1))
            pending = []

    _BUILD_CACHE[key] = nc
    return nc


# ---------------------------------------------------------------------------
# host wrapper
# ---------------------------------------------------------------------------


def kernel(x, b_seq, w1, b1, w2, b2, gamma, beta):
    _install_bir_fix()
    import ml_dtypes
    from concourse.bass_utils import run_bass_kernel_spmd

    bfloat16 = ml_dtypes.bfloat16

    x = np.asarray(x, dtype=np.float32)
    b_seq = np.asarray(b_seq, dtype=np.int32)
    w1 = np.asarray(w1, dtype=np.float32)
    b1 = np.asarray(b1, dtype=np.float32)
    w2 = np.asarray(w2, dtype=np.float32)
    b2 = np.asarray(b2, dtype=np.float32)
    gamma = np.asarray(gamma, dtype=np.float32)
    beta = np.asarray(beta, dtype=np.float32)

    x_flat = x.reshape(NTOK, D_MODEL)
    bs = b_seq.reshape(NTOK)

    # token ids per branch, split evenly over cores
    parts = []  # parts[n][c] -> int array of token ids
    for n in range(1, N_B + 1):
        idx = np.nonzero(bs == n)[0].astype(np.int64)
        parts.append(np.array_split(idx, NCORES))
    caps = []
    for n in range(N_B):
        mx = max(len(p) for p in parts[n])
        caps.append(0 if mx == 0 else ((mx + 127) // 128) * 128)
    S = sum(caps)

    b1_nonzero = bool(np.any(b1))
    b2_nonzero = bool(np.any(b2))
    gb_nontrivial = bool(np.any(beta)) or not bool(np.all(gamma == 1.0))

    nc = _build(tuple(caps), b1_nonzero, b2_nonzero, gb_nontrivial)

    # weight layouts (bf16). W2 is column-centered on the host so the
    # device-side matmul output is already LN-centered.
    w1t = np.ascontiguousarray(
        w1.transpose(0, 2, 1).reshape(N_B, D_MODEL // 128, 128, D_FF)
    ).astype(bfloat16)
    w2b = w2.astype(bfloat16).astype(np.float32)  # [N_B, 256(out j), 1024(f)]
    w2c = w2b - w2b.mean(axis=1, keepdims=True)
    w2t = np.ascontiguousarray(
        w2c.transpose(0, 2, 1).reshape(N_B, D_FF // 128, 128, D_MODEL)
    ).astype(bfloat16)
    if b2_nonzero:
        ncv = np.ascontiguousarray(
            b2 - b2.mean(axis=1, keepdims=True)
        ).astype(bfloat16)

    in_maps = []
    for c in range(NCORES):
        gidx = np.zeros(S, dtype=np.int64)
        seg = 0
        for n in range(N_B):
            p = parts[n][c]
            gidx[seg : seg + len(p)] = p
            seg += caps[n]
        xgc = np.ascontiguousarray(
            x_flat[gidx].T.reshape(D_MODEL // 128, 128, S)
        ).astype(bfloat16)
        m = {"xg": xgc, "w1t": w1t, "w2t": w2t}
        if b2_nonzero:
            m["ncv"] = ncv
        if b1_nonzero:
            m["b1"] = b1.astype(bfloat16)
        if gb_nontrivial:
            m["gamma"] = gamma
            m["beta"] = beta
        in_maps.append(m)

    import os
    import time

    trace = bool(os.environ.get("KERNEL_TRACE"))
    res = None
    for attempt in range(3):
        try:
            res = run_bass_kernel_spmd(
                nc, in_maps, core_ids=list(range(NCORES)), trace=trace
            )
            break
        except Exception:
            # transient NRT device errors have been observed on the first
            # execution of a freshly compiled NEFF; retry
            if attempt == 2:
                raise
            time.sleep(3)
    global LAST_RESULTS
    LAST_RESULTS = res

    out_flat = np.zeros((NTOK, D_MODEL), dtype=np.float32)
    for c in range(NCORES):
        ycc = res.results[c]["yc"]
        seg = 0
        for n in range(N_B):
            p = parts[n][c]
            out_flat[p] = ycc[seg : seg + len(p)]
            seg += caps[n]
    return out_flat.reshape(B, T, D_MODEL)
